# revision 2
# baseline (speedup 1.0000x reference)
"""Chebyshev-KAN (2 layers) Trainium2 kernel, 8-core batch-parallel SPMD.

Math (per layer): per-feature min/max normalize x over the batch to [-1,1],
Chebyshev expansion T_0..T_8, contract: out[b,o] = sum_{i,d} T_d(xn[b,i]) *
coeffs[i,o,d].

Distribution: pure 8-way data-parallel over the batch (b_local = 2048 per
core); every core holds the full coefficient tensors (streamed from HBM).
This removes the hidden-activation AllGather of the 2x4 hybrid entirely:
  - h ([2048 feat, 2048 batch] fp16 = 64KB/partition) stays SBUF-resident
    between the layers; zero DRAM round-trip, zero collective bytes for
    activations.
  - The only communication is per-feature min/max stats: one AllReduce(max)
    over [max | -min] for x, and one tiny AllReduce per L1 output-chunk
    pair, issued as each chunk pair finishes so the last one's ~25us
    latency hides under the first L2 groups (which consume h chunks in
    completion order).
  - The next repeat's x load + stats reduce + AllReduce issue are hoisted
    between L1 and L2 of the current repeat, so in steady state the whole
    prologue (load, reduce, collective latency) overlaps L2 compute and
    the PE never waits at an iteration boundary.

Compute strategy: identical product-basis trick as before
  B1=W1, B2=W1^2, B3=W2*W1, B4=W2^2, B5=W2*B3, B6=B3^2, B7=W4*B3, B8=W4^2
(W_d = 2*T_d(xn)); coefficients re-expressed on the host (exact); d=0 and
basis constants fold into a per-output bias. Basis built 1024 wide
(2 batch blocks per build) to halve fixed per-op overhead; B6 computed on
DVE (tensor_tensor) to balance DVE vs ScalarE occupancy (~55% each).

Matmul structure per layer: groups of (2 output chunks x 4 batch blocks)
= 8 PSUM banks; accumulation over (in_chunks x 8 basis fns) per bank; the
innermost matmul loop runs the 4 batch blocks back-to-back on the same
weight slice so LDWEIGHTS amortizes 4x. Weights are fp16 and each weight
tile is read from HBM exactly once per iteration (67MB/core/iter, ~13%
DMA duty, fully overlapped).
"""

import sys

import numpy as np

try:
    import concourse  # noqa: F401
except ImportError:  # pragma: no cover
    sys.path.insert(0, "/opt/trn_rl_repo")

import concourse.tile as tile  # noqa: E402
from concourse import bacc, mybir  # noqa: E402
from concourse.bass_utils import run_bass_kernel_spmd  # noqa: E402

F32 = mybir.dt.float32
FP16 = mybir.dt.float16
FP16NP = mybir.dt.np(mybir.dt.float16)
ALU = mybir.AluOpType
ACTF = mybir.ActivationFunctionType
AX = mybir.AxisListType

N_CORES = 8
BATCH, IN1, HID, OUT = 16384, 1024, 2048, 1024
BBLK = 512                 # batch block (one PSUM bank of fp32)
OCPG = 2                   # output chunks per matmul group


# --------------------------------------------------------------------------
# host-side math helpers
# --------------------------------------------------------------------------

def _to_product_basis(c: np.ndarray):
    """c [i, o, 9] (T basis) -> p [i, o, 8] (product basis) + kappa [i, o].

    sum_d c_d T_d == sum_k p_k B_k + kappa  (exact, in float64).
    """
    c = c.astype(np.float64)
    c0, c1, c2, c3, c4, c5, c6, c7, c8 = [c[..., d] for d in range(9)]
    p = np.empty(c.shape[:-1] + (8,), np.float64)
    p[..., 0] = 0.5 * (c1 - c3 - c5 + c7)   # B1
    p[..., 1] = 0.5 * (c2 - 3.0 * c6)       # B2
    p[..., 2] = 0.5 * (c3 - c5)             # B3
    p[..., 3] = 0.5 * (c4 - 2.0 * c6)       # B4
    p[..., 4] = 0.5 * (c5 - c7)             # B5
    p[..., 5] = 0.5 * c6                    # B6
    p[..., 6] = 0.5 * c7                    # B7
    p[..., 7] = 0.5 * c8                    # B8
    kappa = c0 - c2 - c4 + 3.0 * c6 - c8
    return p, kappa


def basis_values(xn: np.ndarray) -> list:
    """Reference values of B_1..B_8 for normalized input (for testing)."""
    w1 = 2.0 * xn
    b1 = w1
    b2 = w1 * w1                 # W2 + 2
    b3 = (b2 - 2.0) * b1         # W3 + W1
    b4 = (b2 - 2.0) ** 2         # W4 + 2
    b5 = (b2 - 2.0) * b3         # W5 + W3 + 2W1
    b6 = b3 * b3                 # W6 + 2W4 + 3W2 + 4
    b7 = (b4 - 2.0) * b3         # W7 + W5 + W3 + W1
    b8 = (b4 - 2.0) ** 2         # W8 + 2
    return [b1, b2, b3, b4, b5, b6, b7, b8]


def _pack_weights(p: np.ndarray, n_ic: int) -> np.ndarray:
    """p [I, O_full, 8] -> [128, G*n_ic*8*OCPG*128] fp16.

    SBUF layout: partition = i within 128-chunk; free index =
    ((((g*n_ic + ic)*8 + k)*OCPG + oc)*128 + o), where the output chunk
    index is g*OCPG + oc.
    """
    I, O, K = p.shape
    n_ch = O // 128
    G = n_ch // OCPG
    assert I == n_ic * 128 and K == 8
    a = p.reshape(n_ic, 128, G, OCPG, 128, 8)       # (ic, ii, g, oc, oo, k)
    a = a.transpose(1, 2, 0, 5, 3, 4)               # (ii, g, ic, k, oc, oo)
    return np.ascontiguousarray(a).reshape(128, -1).astype(
        np.float32).astype(FP16NP)


def _pack_bias(bias: np.ndarray) -> np.ndarray:
    """bias [O] -> [128, n_ch] (partition = o within chunk)."""
    n_ch = bias.shape[0] // 128
    return np.ascontiguousarray(bias.reshape(n_ch, 128).T).astype(np.float32)


# --------------------------------------------------------------------------
# device program
# --------------------------------------------------------------------------

def _emit_basis(nc, bpool, src_ap, a_ap, c_ap, n, neg2):
    """Expand one [128, n] chunk into the 8 basis tiles (fp16), returned as
    one [128, 8*n] tile. B1 = a*x + c (per-partition affine) = 2*xn."""
    basis = bpool.tile([128, 8 * n], FP16, tag="basis")

    def b(k):
        return basis[:, (k - 1) * n:k * n]

    sq = ACTF.Square
    nc.vector.tensor_scalar(b(1), src_ap, a_ap, c_ap, ALU.mult, ALU.add)
    nc.scalar.activation(b(2), b(1), sq)
    nc.vector.scalar_tensor_tensor(b(3), b(2), -2.0, b(1), ALU.add, ALU.mult)
    nc.scalar.activation(b(4), b(2), sq, bias=neg2)
    nc.vector.scalar_tensor_tensor(b(5), b(2), -2.0, b(3), ALU.add, ALU.mult)
    nc.vector.tensor_tensor(b(6), b(3), b(3), ALU.mult)
    nc.vector.scalar_tensor_tensor(b(7), b(4), -2.0, b(3), ALU.add, ALU.mult)
    nc.scalar.activation(b(8), b(4), sq, bias=neg2)
    return basis


def _emit_norm_scalars(nc, pool, mx_ap, negmn_ap, out_a, out_c, n):
    """From per-feature max and -min ([128, n]) compute a = 4/rng and
    c = -2*(mx+mn)/rng, so that a*x + c = 2*xn."""
    rng = pool.tile([128, n], F32, tag="normtmp")
    nc.vector.tensor_tensor(rng[:], mx_ap, negmn_ap, ALU.add)        # mx - mn
    rcp = pool.tile([128, n], F32, tag="normtmp")
    nc.vector.reciprocal(rcp[:], rng[:])
    nc.vector.tensor_scalar(out_a, rcp[:], 4.0, None, ALU.mult)      # 4/rng
    s1 = pool.tile([128, n], F32, tag="normtmp")
    nc.vector.tensor_tensor(s1[:], mx_ap, negmn_ap, ALU.subtract)    # mx + mn
    nc.vector.tensor_tensor(s1[:], s1[:], rcp[:], ALU.mult)
    nc.vector.tensor_scalar(out_c, s1[:], -2.0, None, ALU.mult)


def build_bass(batch=BATCH, compile_=True, repeat=1, comm="full", pair=True):
    """Build the SPMD-8 Bass program. `batch` can be scaled down for sim."""
    b_local = batch // N_CORES
    bblk = min(BBLK, b_local)
    n_blk = b_local // bblk                          # 4 at full size
    assert OCPG * n_blk <= 8
    BP = min(2, n_blk)                               # blocks per basis build
    PW = BP * bblk                                   # basis build width
    n_ic1, n_ch1 = IN1 // 128, HID // 128            # 8, 16
    n_ic2, n_ch2 = HID // 128, OUT // 128            # 16, 8
    G1, G2 = n_ch1 // OCPG, n_ch2 // OCPG            # 8, 4
    seg1 = 8 * OCPG * 128                            # w cols per (g, ic)
    seg2 = 8 * OCPG * 128

    nc = bacc.Bacc("TRN2", target_bir_lowering=False, debug=False,
                   num_devices=N_CORES)

    x_d = nc.dram_tensor("x", [IN1, b_local], FP16, kind="ExternalInput").ap()
    w1 = nc.dram_tensor("w1", [128, G1 * n_ic1 * seg1], FP16,
                        kind="ExternalInput").ap()
    w2 = nc.dram_tensor("w2", [128, G2 * n_ic2 * seg2], FP16,
                        kind="ExternalInput").ap()
    bias1_d = nc.dram_tensor("bias1", [128, n_ch1], F32,
                             kind="ExternalInput").ap()
    bias2_d = nc.dram_tensor("bias2", [128, n_ch2], F32,
                             kind="ExternalInput").ap()
    y = nc.dram_tensor("y", [OUT, b_local], F32, kind="ExternalOutput").ap()

    grp_all = [list(range(N_CORES))]

    with tile.TileContext(nc) as tc:
        with (
            tc.tile_pool(name="const", bufs=1) as cpool,
            tc.tile_pool(name="xin", bufs=1) as xpool,
            tc.tile_pool(name="hsb", bufs=1) as hpool,
            tc.tile_pool(name="w", bufs=6) as wpool,
            tc.tile_pool(name="basis", bufs=4) as bpool,
            tc.tile_pool(name="ysb", bufs=4) as ypool,
            tc.tile_pool(name="small", bufs=2) as spool,
            tc.tile_pool(name="stmp", bufs=4) as stpool,
            tc.tile_pool(name="acc", bufs=8, space="PSUM") as ppool,
            tc.tile_pool(name="dstat", bufs=6, space="DRAM") as dspool,
        ):
            # ---- loop-invariant constants ----
            neg2t = cpool.tile([128, 1], F32)
            nc.vector.memset(neg2t[:], -2.0)
            b1sb = cpool.tile([128, n_ch1], F32)
            nc.sync.dma_start(b1sb[:], bias1_d[:])
            b2sb = cpool.tile([128, n_ch2], F32)
            nc.sync.dma_start(b2sb[:], bias2_d[:])

            def emit_xload_stats():
                """Load x (fp16, feature-major) into SBUF, reduce per-feature
                max/-min, stage + issue the global AllReduce(max)."""
                # x DMAs ride the ACT ring: the sync ring carries the weight
                # stream, and a hoisted x load sitting ahead of L2's weight
                # DMAs there would stall the next layer's prefetch.
                x_sb = xpool.tile([128, n_ic1 * b_local], FP16, tag="x")
                for ic in range(n_ic1):
                    nc.scalar.dma_start(
                        x_sb[:, ic * b_local:(ic + 1) * b_local],
                        x_d[ic * 128:(ic + 1) * 128, :])
                st1 = spool.tile([128, 2 * n_ic1], F32, tag="st1")
                for ic in range(n_ic1):
                    xc = x_sb[:, ic * b_local:(ic + 1) * b_local]
                    nc.vector.tensor_reduce(st1[:, ic:ic + 1], xc, AX.X,
                                            ALU.max)
                    mn = stpool.tile([128, 1], F32, tag="mn")
                    nc.vector.tensor_reduce(mn[:], xc, AX.X, ALU.min)
                    nc.vector.tensor_scalar(
                        st1[:, n_ic1 + ic:n_ic1 + ic + 1],
                        mn[:], -1.0, None, ALU.mult)
                ar_in = dspool.tile([128, 2 * n_ic1], F32, tag="dstat")
                # AllReduce outputs are read late (during L2) — give them a
                # deep ring so later stats writes never WAR-chain onto them.
                ar_out = dspool.tile([128, 2 * n_ic1], F32, tag="dstat_out",
                                     bufs=12)
                nc.gpsimd.dma_start(ar_in[:], st1[:])
                nc.gpsimd.collective_compute(
                    "AllReduce", ALU.max, replica_groups=grp_all,
                    ins=[ar_in.opt()], outs=[ar_out.opt()])
                return x_sb, ar_out

            def emit_stats_finish(ar_out):
                """Load the AllReduce result and derive a1/c1."""
                stg = spool.tile([128, 2 * n_ic1], F32, tag="stg")
                nc.scalar.dma_start(stg[:], ar_out[:])
                a1 = spool.tile([128, n_ic1], F32, tag="a1")
                c1 = spool.tile([128, n_ic1], F32, tag="c1")
                _emit_norm_scalars(nc, stpool, stg[:, 0:n_ic1],
                                   stg[:, n_ic1:], a1[:], c1[:], n_ic1)
                return a1, c1

            for _rep in range(repeat):
                if _rep == 0:
                    x_sb, x_ar = emit_xload_stats()
                a1, c1 = emit_stats_finish(x_ar)

                # ======== L1: h = cheby_layer(x) , feature chunks 0..15 ====
                hmax = spool.tile([128, n_ch1], F32, tag="hmax")
                hmin = spool.tile([128, n_ch1], F32, tag="hmin")
                h_sb = hpool.tile([128, n_ch1 * b_local], FP16, tag="h")
                h_ar = []

                def l1_epilogue(g, ps):
                    """Drain the group's PSUM banks into resident h (bias
                    add), update running stats, stage + AllReduce the now-
                    final chunk-pair stats. Emitted right after the group's
                    matmuls: the per-bank stop semaphores fire staggered
                    during the group's MM tail, so the drains already
                    overlap the tail in the ACT FIFO."""
                    for oc in range(OCPG):
                        ch = g * OCPG + oc
                        for blk in range(n_blk):
                            hsl = h_sb[:, ch * b_local + blk * bblk:
                                       ch * b_local + (blk + 1) * bblk]
                            nc.scalar.activation(hsl, ps[oc][blk][:],
                                                 ACTF.Identity,
                                                 bias=b1sb[:, ch:ch + 1])
                            if blk == 0:
                                nc.vector.tensor_reduce(hmax[:, ch:ch + 1],
                                                        hsl, AX.X, ALU.max)
                                nc.vector.tensor_reduce(hmin[:, ch:ch + 1],
                                                        hsl, AX.X, ALU.min)
                            else:
                                tmx = stpool.tile([128, 1], F32, tag="tmx")
                                nc.vector.tensor_reduce(tmx[:], hsl, AX.X,
                                                        ALU.max)
                                nc.vector.tensor_tensor(hmax[:, ch:ch + 1],
                                                        hmax[:, ch:ch + 1],
                                                        tmx[:], ALU.max)
                                tmn = stpool.tile([128, 1], F32, tag="tmn")
                                nc.vector.tensor_reduce(tmn[:], hsl, AX.X,
                                                        ALU.min)
                                nc.vector.tensor_tensor(hmin[:, ch:ch + 1],
                                                        hmin[:, ch:ch + 1],
                                                        tmn[:], ALU.min)
                    # stage + AllReduce this group's chunk-pair stats now;
                    # the L2 ic-loop consumes them in the same order, so
                    # even the last one's latency hides under L2 compute.
                    sgt = stpool.tile([128, 2 * OCPG], F32, tag="sgt")
                    nc.vector.tensor_copy(
                        sgt[:, 0:OCPG], hmax[:, g * OCPG:(g + 1) * OCPG])
                    nc.vector.tensor_scalar(
                        sgt[:, OCPG:], hmin[:, g * OCPG:(g + 1) * OCPG],
                        -1.0, None, ALU.mult)
                    hin = dspool.tile([128, 2 * OCPG], F32, tag="dstat")
                    hout = dspool.tile([128, 2 * OCPG], F32, tag="dstat_out2",
                                       bufs=12)
                    nc.gpsimd.dma_start(hin[:], sgt[:])
                    nc.gpsimd.collective_compute(
                        "AllReduce", ALU.max, replica_groups=grp_all,
                        ins=[hin.opt()], outs=[hout.opt()])
                    h_ar.append(hout)

                for g in range(G1):
                    ps = [[ppool.tile([128, bblk], F32, tag="acc",
                                      name=f"ps1_{_rep}_{g}_{oc}_{blk}")
                           for blk in range(n_blk)] for oc in range(OCPG)]
                    for ic in range(n_ic1):
                        wt = wpool.tile([128, seg1], FP16, tag="w")
                        off_w = (g * n_ic1 + ic) * seg1
                        nc.sync.dma_start(wt[:], w1[:, off_w:off_w + seg1])
                        bas = []
                        for bp in range(n_blk // BP):
                            bas.append(_emit_basis(
                                nc, bpool,
                                x_sb[:, ic * b_local + bp * PW:
                                     ic * b_local + (bp + 1) * PW],
                                a1[:, ic:ic + 1], c1[:, ic:ic + 1], PW,
                                neg2t[:]))
                        for k in range(8):
                            for oc in range(OCPG):
                                woff = (k * OCPG + oc) * 128
                                for blk in range(n_blk):
                                    bp, j = divmod(blk, BP)
                                    nc.tensor.matmul(
                                        ps[oc][blk][:],
                                        wt[:, woff:woff + 128],
                                        bas[bp][:, k * PW + j * bblk:
                                                k * PW + j * bblk + bblk],
                                        start=(ic == 0 and k == 0),
                                        stop=(ic == n_ic1 - 1 and k == 7))
                    l1_epilogue(g, ps)

                # hoisted next-iteration prologue: its DMA + DVE reduces and
                # the collective all overlap this iteration's L2.
                if _rep + 1 < repeat:
                    x_sb, x_ar = emit_xload_stats()

                # ======== L2: y = cheby_layer(h), streaming a2/c2 ========
                a2 = spool.tile([128, n_ic2], F32, tag="a2")
                c2 = spool.tile([128, n_ic2], F32, tag="c2")
                a2_done = set()

                def finish_a2(g1):
                    hl = stpool.tile([128, 2 * OCPG], F32, tag="hl")
                    nc.scalar.dma_start(hl[:], h_ar[g1][:])
                    _emit_norm_scalars(
                        nc, stpool, hl[:, 0:OCPG], hl[:, OCPG:],
                        a2[:, g1 * OCPG:(g1 + 1) * OCPG],
                        c2[:, g1 * OCPG:(g1 + 1) * OCPG], OCPG)
                    a2_done.add(g1)

                def l2_epilogue(g, ps):
                    for oc in range(OCPG):
                        ch = g * OCPG + oc
                        for blk in range(n_blk):
                            ysb = ypool.tile([128, bblk], F32, tag="y")
                            nc.scalar.activation(ysb[:], ps[oc][blk][:],
                                                 ACTF.Identity,
                                                 bias=b2sb[:, ch:ch + 1])
                            nc.scalar.dma_start(
                                y[ch * 128:(ch + 1) * 128,
                                  blk * bblk:(blk + 1) * bblk], ysb[:])

                for g in range(G2):
                    ps = [[ppool.tile([128, bblk], F32, tag="acc",
                                      name=f"ps2_{_rep}_{g}_{oc}_{blk}")
                           for blk in range(n_blk)] for oc in range(OCPG)]
                    for ic in range(n_ic2):
                        if ic // OCPG not in a2_done:
                            finish_a2(ic // OCPG)
                        wt = wpool.tile([128, seg2], FP16, tag="w")
                        off_w = (g * n_ic2 + ic) * seg2
                        nc.sync.dma_start(wt[:], w2[:, off_w:off_w + seg2])
                        bas = []
                        for bp in range(n_blk // BP):
                            bas.append(_emit_basis(
                                nc, bpool,
                                h_sb[:, ic * b_local + bp * PW:
                                     ic * b_local + (bp + 1) * PW],
                                a2[:, ic:ic + 1], c2[:, ic:ic + 1], PW,
                                neg2t[:]))
                        for k in range(8):
                            for oc in range(OCPG):
                                woff = (k * OCPG + oc) * 128
                                for blk in range(n_blk):
                                    bp, j = divmod(blk, BP)
                                    nc.tensor.matmul(
                                        ps[oc][blk][:],
                                        wt[:, woff:woff + 128],
                                        bas[bp][:, k * PW + j * bblk:
                                                k * PW + j * bblk + bblk],
                                        start=(ic == 0 and k == 0),
                                        stop=(ic == n_ic2 - 1 and k == 7))
                    l2_epilogue(g, ps)

    if compile_:
        nc.compile()
    return nc


# --------------------------------------------------------------------------
# host wrapper
# --------------------------------------------------------------------------

def prepare_in_maps(x, coeffs1, coeffs2, batch=BATCH):
    xT = np.ascontiguousarray(x.T).astype(FP16NP)        # [IN1, batch] fp16
    b_local = batch // N_CORES

    p1, k1 = _to_product_basis(coeffs1)                  # [IN1, HID, 8]
    p2, k2 = _to_product_basis(coeffs2)
    bias1 = k1.sum(axis=0)                               # [HID]
    bias2 = k2.sum(axis=0)                               # [OUT]

    w1p = _pack_weights(p1, IN1 // 128)
    w2p = _pack_weights(p2, HID // 128)
    b1p = _pack_bias(bias1)
    b2p = _pack_bias(bias2)

    in_maps = []
    for core in range(N_CORES):
        in_maps.append(dict(
            x=np.ascontiguousarray(
                xT[:, core * b_local:(core + 1) * b_local]),
            w1=w1p, w2=w2p, bias1=b1p, bias2=b2p,
        ))
    return in_maps


def assemble_output(results, batch=BATCH):
    b_local = batch // N_CORES
    y = np.empty((batch, OUT), np.float32)
    for core in range(N_CORES):
        y[core * b_local:(core + 1) * b_local, :] = results[core]["y"].T
    return y


def _disable_birsim():
    """Skip walrus's BIR-simulator pass (compile-time only; big speedup).
    Optionally (KAN_LDW_OPT=1) enable walrus's ldweights optimization."""
    import os
    import concourse.bass_utils as bu
    if getattr(bu, "_birsim_patched", False):
        return
    orig = bu.run_command
    ldw = os.environ.get("KAN_LDW_OPT") == "1"

    def patched(cmd, **kw):
        def fix(c):
            if not isinstance(c, str):
                return c
            c = c.replace("--enable-birsim=true", "--enable-birsim=false")
            if ldw:
                c = c.replace("--enable-ldw-opt=false", "--enable-ldw-opt=true")
            return c
        return orig([fix(c) for c in cmd], **kw)

    bu.run_command = patched
    bu._birsim_patched = True


_NC_CACHE = {}


def kernel(x, coeffs1, coeffs2):
    assert x.shape == (BATCH, IN1), x.shape
    _disable_birsim()
    if "nc" not in _NC_CACHE:
        _NC_CACHE["nc"] = build_bass(BATCH)
    nc = _NC_CACHE["nc"]
    in_maps = prepare_in_maps(np.asarray(x, np.float32),
                              np.asarray(coeffs1, np.float32),
                              np.asarray(coeffs2, np.float32), BATCH)
    res = run_bass_kernel_spmd(nc, in_maps, core_ids=list(range(N_CORES)))
    return assemble_output(res.results, BATCH)
